# revision 76
# baseline (speedup 1.0000x reference)
"""DigitCaps dynamic-routing kernel for 8 Trainium2 NeuronCores — v5.

Math (reference):
  u_hat[b,i,j,d] = sum_e W[0,i,j,d,e] * x[b,i,e]
  2 routing iterations; iteration 1 has b=0 so c = 1/32 exactly:
    s1 = (1/32) sum_i u_hat ;  v1 = squash(s1)
    b2 = sum_d u_hat * v1    ;  c2 = softmax_j(b2)
    s2 = sum_i c2 * u_hat    ;  v2 = squash(s2)   -> output

Structure (per core, i sharded 8 ways, batch b=128):
  s1: K=2048 GEMM in 4 column blocks; AllReduce + squash + v1-transpose
      pipelined per 128-col block so pass 2a starts early.
  pass 2 runs in TRANSPOSED land, u_hat never materialized:
    A_T[(i16,e8),q,b] = sum_d W[i,j,d,e] v1[b,j,d]    (8 K=16 GEMMs/chunk)
    P_T = A_T * xT                                    (one mult/chunk)
    b2T[i,b] = sum_{(i16,e8)} R_q * P_T               (8 K=128 GEMMs per
                                                       4-chunk group, N=512,
                                                       PSUM-accumulated; R_q is
                                                       a constant 0/1 matrix —
                                                       the e-reduce runs on PE)
    eeT = exp(b2T)    (ACT exp on PSUM->SBUF copy-out, batched 4 j's)
    zT,rzT = softmax denom (incremental partials per 8 j's), x2t = xT*rzT
    yT[(i),j,b] = x2t * eeT                           (broadcast mult)
    s2[b,(j,d)] = sum_{kt=(e,h)} sum_i yT * W         (per-(j,kt) GEMM, N=16)
  s2 drains in 4 column blocks (copy+AllReduce+squash+out per block).
  Tail (odd-kt yT after rz[1]) splits each mult DVE/Pool by batch columns.
"""

import sys
for _p in ("/opt/pypackages", "/opt/trn_rl_repo"):
    if _p not in sys.path:
        sys.path.insert(0, _p)

import numpy as np
import ml_dtypes

import concourse.bass as bass
import concourse.bacc as bacc
import concourse.tile as tile
from concourse import mybir
from concourse.bass_utils import run_bass_kernel_spmd

B = 128
I = 2048
E = 8
J = 32
D = 16
JD = J * D          # 512
NC_ = 8             # cores
IS = I // NC_       # 256 in_caps per core
EPS = 1e-8

f32 = mybir.dt.float32
bf16 = mybir.dt.bfloat16
USE_F8 = True    # fp8 for the routing-path weights (w1/wt/xp8/v1t2)
f8 = mybir.dt.float8e4 if USE_F8 else bf16
# fp8e4m3 min-normal is 2^-6: W (~0.01) and v1 (~0.01..0.2) must be scaled
# into the normal range, compensated at the s1c copy and the exp
W8SCALE = 64.0 if USE_F8 else 1.0         # host multiplier on wt
W1SCALE = 64.0 if USE_F8 else (1.0 / J)   # host multiplier on w1
S1CSCALE = 1.0 / (J * 64.0) if USE_F8 else 1.0
V1TSCALE = 16.0 if USE_F8 else 1.0        # ACT scale on the v1t2 copy
EXPSCALE = 1.0 / (64.0 * 16.0) if USE_F8 else 1.0

# ---- engine assignment knobs (tuned against TimelineSim) ----
# 64 chunks (chunk = h*32 + j): who moves A_T from PSUM and multiplies by x:
#  'act'  = ACT copies PSUM->bf16 SBUF, DVE does the 2x mult
#  'dve'  = DVE mult direct from PSUM (1x)
#  'pool' = Pool mult direct from PSUM
def _ppath(c):
    # No 'pool' mults: Pool's 2127ns latency in the R dependency chain
    # costs more than its throughput adds (Pool still runs yT b-splits).
    if c < 32:   # sweep 1
        if c < 12:
            return 'dve' if c % 2 == 1 else 'act'
        return ('act', 'act', 'dve', 'act')[c % 4]
    if c >= 62:  # end-game: skip the 2-stage ACT path on the last chunks
        return 'dve'
    return 'act'  # sweep 2
P_PATH = [_ppath(c) for c in range(64)]
# yT mults: every quarter-unit is split DVE/Pool by batch columns
YT_BSPLIT = 92      # sweep-2 even kt: DVE b[0:92], Pool b[92:128]
TAIL_BSPLIT = 104   # tail odd kt: DVE b[0:104], Pool b[104:128]


def _bc(ap, n):
    """Broadcast an AP along a new innermost dim of size n (step 0)."""
    return bass.AP(tensor=ap.tensor, offset=ap.offset, ap=[*ap.ap, [0, n]])


def _bc_mid(ap, n):
    """Broadcast an AP along a new dim of size n inserted before the last
    free dim (step 0)."""
    return bass.AP(tensor=ap.tensor, offset=ap.offset,
                   ap=[*ap.ap[:-1], [0, n], ap.ap[-1]])


def _strided(ap, dims):
    """Replace the free dims of a [P, 1]-sliced AP with custom [step, num]
    pairs (partition dim kept)."""
    return bass.AP(tensor=ap.tensor, offset=ap.offset, ap=[ap.ap[0], *dims])


def _squash_full(nc, pool, s_sb, v_sb):
    """v = |s|^2/(1+|s|^2) * s/(|s|+eps) over all 32 j at once."""
    sq = pool.tile([B, JD], f32, tag="fsqs")
    nc.vector.tensor_mul(sq[:, :], s_sb[:, :], s_sb[:, :])
    n2 = pool.tile([B, J], f32, tag="fsqn2")
    nc.vector.tensor_reduce(
        out=n2[:, :], in_=sq[:, :].rearrange("p (j d) -> p j d", d=D),
        axis=mybir.AxisListType.X, op=mybir.AluOpType.add)
    nrm = pool.tile([B, J], f32, tag="fsqn")
    nc.scalar.sqrt(out=nrm[:, :], in_=n2[:, :])
    a1 = pool.tile([B, J], f32, tag="fsqa")
    nc.vector.tensor_scalar_add(a1[:, :], n2[:, :], 1.0)
    b1 = pool.tile([B, J], f32, tag="fsqb")
    nc.vector.tensor_scalar_add(b1[:, :], nrm[:, :], EPS)
    den = pool.tile([B, J], f32, tag="fsqd")
    nc.vector.tensor_mul(den[:, :], a1[:, :], b1[:, :])
    rden = pool.tile([B, J], f32, tag="fsqr")
    nc.vector.reciprocal(out=rden[:, :], in_=den[:, :])
    sc = pool.tile([B, J], f32, tag="fsqc")
    nc.vector.tensor_mul(sc[:, :], n2[:, :], rden[:, :])
    nc.vector.tensor_tensor(
        out=v_sb[:, :].rearrange("p (j d) -> p j d", d=D),
        in0=s_sb[:, :].rearrange("p (j d) -> p j d", d=D),
        in1=_bc(sc[:, :], D), op=mybir.AluOpType.mult)


def _squash_blk(nc, pool, s_sb, v_sb, t, tag):
    """v = |s|^2/(1+|s|^2) * s/(|s|+eps) for j-block t (8 j's, 128 cols)."""
    nj = J // 4
    c0 = 128 * t
    sv = s_sb[:, c0:c0 + 128]
    sq = pool.tile([B, 128], f32, tag=f"sqs{tag}", name=f"sqs{tag}{t}")
    nc.vector.tensor_mul(sq[:, :], sv, sv)
    n2 = pool.tile([B, nj], f32, tag=f"sqn2{tag}", name=f"sqn2{tag}{t}")
    nc.vector.tensor_reduce(
        out=n2[:, :], in_=sq[:, :].rearrange("p (j d) -> p j d", d=D),
        axis=mybir.AxisListType.X, op=mybir.AluOpType.add)
    nrm = pool.tile([B, nj], f32, tag=f"sqn{tag}", name=f"sqn{tag}{t}")
    nc.scalar.sqrt(out=nrm[:, :], in_=n2[:, :])
    a1 = pool.tile([B, nj], f32, tag=f"sqa{tag}", name=f"sqa{tag}{t}")
    nc.vector.tensor_scalar_add(a1[:, :], n2[:, :], 1.0)
    b1 = pool.tile([B, nj], f32, tag=f"sqb{tag}", name=f"sqb{tag}{t}")
    nc.vector.tensor_scalar_add(b1[:, :], nrm[:, :], EPS)
    den = pool.tile([B, nj], f32, tag=f"sqd{tag}", name=f"sqd{tag}{t}")
    nc.vector.tensor_mul(den[:, :], a1[:, :], b1[:, :])
    rden = pool.tile([B, nj], f32, tag=f"sqr{tag}", name=f"sqr{tag}{t}")
    nc.vector.reciprocal(out=rden[:, :], in_=den[:, :])
    sc = pool.tile([B, nj], f32, tag=f"sqc{tag}", name=f"sqc{tag}{t}")
    nc.vector.tensor_mul(sc[:, :], n2[:, :], rden[:, :])
    nc.vector.tensor_tensor(
        out=v_sb[:, c0:c0 + 128].rearrange("p (j d) -> p j d", d=D),
        in0=sv.rearrange("p (j d) -> p j d", d=D),
        in1=_bc(sc[:, :], D), op=mybir.AluOpType.mult)


def build_nc(num_devices=NC_, with_cc=True):
    nc = bacc.Bacc("TRN2", target_bir_lowering=False, debug=False,
                   num_devices=num_devices)
    # host-prepped per-core inputs (see _prep_inputs for layouts)
    # w1/wt/xp8 feed only the routing-coefficient path (b2 is ~7e-4, so
    # exp(b2)~1+b2 and softmax damps any error there by ~1e3x): fp8 is safe
    xt_d = nc.dram_tensor("xt", [128, 16, B], bf16, kind="ExternalInput")
    xt2_d = nc.dram_tensor("xt2", [128, 16, B], bf16, kind="ExternalInput")
    xp8_d = nc.dram_tensor("xp8", [128, 16, B], f8, kind="ExternalInput")
    w1_d = nc.dram_tensor("w1", [128, 4, 16, 128], f8, kind="ExternalInput")
    # wt rows = 32*(j%4) + d (d<16 real, 16..31 zero-padded): K=32 matmuls
    # with lhsT/rhs/tile_position all 32-aligned (Ldweights requires it)
    wt_d = nc.dram_tensor("wt", [128, 8, 2, 8, 128], f8, kind="ExternalInput")
    ws_d = nc.dram_tensor("ws", [128, J, 16, D], bf16, kind="ExternalInput")
    r2_d = nc.dram_tensor("r2", [128, 8, 128], bf16, kind="ExternalInput")
    idf_d = nc.dram_tensor("identf", [128, 128], f32, kind="ExternalInput")
    yout = nc.dram_tensor("yout", [B, JD], f32, kind="ExternalOutput")
    dscr_o = nc.dram_tensor("dscr", [B, 1], f32, kind="ExternalOutput")

    cc1i = [nc.dram_tensor(f"cc1i{t}", [B, 128], f32) for t in range(4)]
    cc1o = [nc.dram_tensor(f"cc1o{t}", [B, 128], f32, addr_space="Shared")
            for t in range(4)]
    cc2i = [nc.dram_tensor(f"cc2i{t}", [B, 128], f32) for t in range(4)]
    cc2o = [nc.dram_tensor(f"cc2o{t}", [B, 128], f32, addr_space="Shared")
            for t in range(4)]
    rgroups = [list(range(num_devices))]

    def allreduce(cin, cout, eng=None):
        if with_cc:
            nc.gpsimd.collective_compute(
                "AllReduce", mybir.AluOpType.add, replica_groups=rgroups,
                ins=[cin.ap()], outs=[cout.ap()])
        else:
            (eng or nc.sync).dma_start(out=cout.ap(), in_=cin.ap())

    with tile.TileContext(nc) as tc:
        with (
            tc.tile_pool(name="consts", bufs=1) as consts,
            tc.tile_pool(name="work", bufs=1) as work,
            tc.tile_pool(name="latep", bufs=1) as latep,
            tc.tile_pool(name="ptg", bufs=3) as ptgpool,
            tc.tile_pool(name="pacp", bufs=5) as pacp,
            tc.tile_pool(name="ypool", bufs=2) as ypool,
            tc.tile_pool(name="pa", bufs=3, space="PSUM") as pa_pool,
            tc.tile_pool(name="pb", bufs=1, space="PSUM") as pb_pool,
            tc.tile_pool(name="pacc", bufs=1, space="PSUM") as pacc,
        ):
            # ---------- input DMAs (ordered by first use) ----------
            gd_xt2 = latep.tile([128, 16, B], bf16, tag="lxt2",
                                name="gd_xt2")
            gd_r2 = latep.tile([128, 8, 128], bf16, tag="lr2", name="gd_r2")
            gd_wt = latep.tile([128, 7, 2, 8, 128], f8, tag="lwt",
                               name="gd_wt")
            gd_ws = latep.tile([128, J, 16, D], bf16, tag="lws",
                               name="gd_ws")
            nc.vector.memset(gd_xt2[:, 0:1, 0:1], 0.0)
            nc.vector.memset(gd_r2[:, 0:1, 0:1], 0.0)
            nc.vector.memset(gd_wt[:, 0:1, 0:1, 0:1, 0:1], 0.0)
            nc.vector.memset(gd_ws[:, 0:1, 0:1, 0:1], 0.0)
            identf = consts.tile([128, 128], f32, tag="identf")
            nc.sync.dma_start(out=identf[:, :], in_=idf_d.ap())
            xt = consts.tile([128, 16, B], bf16, tag="xt")
            nc.sync.dma_start(out=xt[:, :, :], in_=xt_d.ap())
            xp8 = consts.tile([128, 16, B], f8, tag="xp8")
            nc.sync.dma_start(out=xp8[:, :, :], in_=xp8_d.ap())
            w1 = consts.tile([128, 4, 16, 128], f8, tag="w1")
            for t in range(4):   # column blocks to pipeline pass 1
                nc.sync.dma_start(out=w1[:, t, :, :],
                                  in_=w1_d.ap()[:, t, :, :])
            wt0 = consts.tile([128, 1, 2, 8, 128], f8, tag="wt0")
            nc.sync.dma_start(out=wt0[:, 0, :, :, :],
                              in_=wt_d.ap()[:, 0, :, :, :])

            # ---------- pass 1 + AR1 + squash + v1T, per 128-col block ----
            # each 128-col block accumulates in its OWN psum tile (from the
            # pa ring) because PSUM dependencies are tile-granular: a shared
            # [B,512] tile would stall every copy until the last matmul
            ps1b = []
            s1c = work.tile([B, JD], f32, tag="s1c")
            s1 = consts.tile([B, JD], f32, tag="s1")
            v1 = consts.tile([B, JD], f32, tag="v1")
            # v1t2[32*(j%4)+d, j//4, b] = v1[b, 16j+d] (rows 16..31 zero)
            v1t2 = consts.tile([128, 8, B], f8, tag="v1t2")
            v1pad = work.tile([B, 8, 128], f32, tag="v1pad")
            nc.vector.memset(v1pad[:, :, :], 0.0)
            for t in range(4):
                pt1 = pa_pool.tile([B, 8, 128], f32, tag="pa",
                                   name=f"ps1b{t}")
                ps1b.append(pt1)
                for kt in range(16):
                    nc.tensor.matmul(
                        out=pt1[:, 0, :],
                        lhsT=xp8[:, kt, :],
                        rhs=w1[:, t, kt, :],
                        start=(kt == 0), stop=(kt == 15))
                nc.scalar.activation(
                    out=s1c[:, 128 * t:128 * t + 128], in_=pt1[:, 0, :],
                    func=mybir.ActivationFunctionType.Copy, scale=S1CSCALE)
            # AR chains ride the two HWDGE queues (SP even blocks, ACT odd)
            # so they dispatch in parallel and their small DMAs claim the
            # DMA engines before the late input loads
            for t in range(4):
                eng = [nc.sync, nc.scalar, nc.sync, nc.scalar][t]
                eng.dma_start(out=cc1i[t].ap(),
                              in_=s1c[:, 128 * t:128 * t + 128])
                allreduce(cc1i[t], cc1o[t], eng=eng)
                eng.dma_start(out=s1[:, 128 * t:128 * t + 128],
                              in_=cc1o[t].ap())
                _squash_blk(nc, work, s1, v1, t, "v1")
            # late inputs are WAR-gated: their tiles reuse pool buffers
            # whose previous tile is read by an s1-dependent op, so the DMAs
            # carry a real semaphore and the DMA-device FIFO (ordered by
            # request time) serves the AR-chain hops first
            gr = work.tile([B, 1], f32, tag="gater")
            nc.vector.tensor_tensor(out=gr[:, :], in0=gd_xt2[:, 0:1, 0],
                                    in1=s1[:, 128:129],
                                    op=mybir.AluOpType.add)
            nc.vector.tensor_tensor(out=gr[:, :], in0=gd_wt[:, 0:1, 0, 0, 0],
                                    in1=s1[:, 128:129],
                                    op=mybir.AluOpType.add)
            nc.vector.tensor_tensor(out=gr[:, :], in0=gd_ws[:, 0:1, 0, 0],
                                    in1=s1[:, 128:129],
                                    op=mybir.AluOpType.add)
            nc.vector.tensor_tensor(out=gr[:, :], in0=gd_r2[:, 0:1, 0],
                                    in1=s1[:, 128:129],
                                    op=mybir.AluOpType.add)
            xt2 = latep.tile([128, 16, B], bf16, tag="lxt2")
            nc.gpsimd.dma_start(out=xt2[:, :, :], in_=xt2_d.ap())
            r2 = latep.tile([128, 8, 128], bf16, tag="lr2")
            nc.gpsimd.dma_start(out=r2[:, :, :], in_=r2_d.ap())
            wt = latep.tile([128, 7, 2, 8, 128], f8, tag="lwt")
            for g in range(7):
                nc.gpsimd.dma_start(out=wt[:, g, :, :, :],
                                    in_=wt_d.ap()[:, g + 1, :, :, :])
            ws = latep.tile([128, J, 16, D], bf16, tag="lws")
            for g in range(4):
                nc.gpsimd.dma_start(
                    out=ws[:, 8 * g:8 * g + 8, :, :],
                    in_=ws_d.ap()[:, 8 * g:8 * g + 8, :, :])

            def emit_v1T(t):
                """Transpose v1 block t (j in [8t, 8t+8)) into padded 32-row
                slots of v1t2 (deferred into the chunk loop so it doesn't
                head-of-line block the PE queue)."""
                # v1pad[b, tt, 32s+d] = v1[b, 64tt+16s+d] for the 2 tt's
                for s in range(4):
                    nc.vector.tensor_copy(
                        _strided(v1pad[:, 2 * t, 32 * s:32 * s + 1],
                                 [[128, 2], [1, 16]]),
                        _strided(v1[:, 128 * t + 16 * s:128 * t + 16 * s + 1],
                                 [[64, 2], [1, 16]]))
                tpv = pacc.tile([128, 4, 128], f32, tag="acc",
                                name=f"tpv{t}")
                for k in range(2):
                    nc.tensor.transpose(
                        out=tpv[:, k, :],
                        in_=v1pad[:, 2 * t + k, :], identity=identf[:, :])
                nc.scalar.activation(
                    out=v1t2[:, 2 * t:2 * t + 2, :], in_=tpv[:, 0:2, :],
                    func=mybir.ActivationFunctionType.Copy, scale=V1TSCALE)

            # ---------- pass 2a state ----------
            eeT = [consts.tile([128, J, B], bf16, tag=f"eeT{h}",
                               name=f"eeT{h}") for h in range(2)]
            zp = [[None] * 4, [None] * 4]   # partial z per 8-j group
            rz = [None, None]
            x2t = consts.tile([128, 16, B], bf16, tag="x2t")
            ps2 = pacc.tile([B, JD], f32, tag="acc", name="ps2")

            def emit_AT(h, j):
                """8 K=32 GEMMs (16 zero-pad rows): pa[:, q, :] =
                W_hjq^T . v1_j^T; returns pa."""
                pa = pa_pool.tile([B, 8, 128], f32, tag="pa",
                                  name=f"pa{h}_{j}")
                s = j % 4
                g = j // 4
                wsrc = wt0[32 * s:32 * s + 32, 0, h, :, :] if g == 0 else \
                    wt[32 * s:32 * s + 32, g - 1, h, :, :]
                for q in range(8):
                    nc.tensor.matmul(
                        out=pa[:, q, :],
                        lhsT=wsrc[:, q, :],
                        rhs=v1t2[32 * s:32 * s + 32, j // 4, :],
                        start=True, stop=True,
                        tile_position=(32 * s, 0))
                return pa

            def emit_mult(h, j, pa, ptgt):
                """P_T mult into ptgt[:, j%4, :, :] via the chunk's engine."""
                chunk = h * J + j
                xs = xt2[:, 8 * h:8 * h + 8, :]
                out = ptgt[:, j % 4, :, :]
                path = P_PATH[chunk]
                if path == 'act':
                    pac = pacp.tile([128, 8, 128], bf16, tag="pac",
                                    name=f"pac{chunk}")
                    nc.scalar.copy(out=pac[:, :, :], in_=pa[:, :, :])
                    nc.vector.tensor_tensor(
                        out=out, in0=pac[:, :, :], in1=xs,
                        op=mybir.AluOpType.mult)
                elif path == 'pcopy':
                    pac = pacp.tile([128, 8, 128], bf16, tag="pac",
                                    name=f"pac{chunk}")
                    nc.gpsimd.tensor_copy(pac[:, :, :], pa[:, :, :])
                    nc.vector.tensor_tensor(
                        out=out, in0=pac[:, :, :], in1=xs,
                        op=mybir.AluOpType.mult)
                elif path == 'dve':
                    nc.vector.tensor_tensor(
                        out=out, in0=pa[:, :, :], in1=xs,
                        op=mybir.AluOpType.mult)
                else:
                    nc.gpsimd.tensor_tensor(
                        out=out, in0=pa[:, :, :], in1=xs,
                        op=mybir.AluOpType.mult)

            def emit_R(h, g, ptgt):
                """Batched e-reduce for 4 chunks; returns the psum tile."""
                pb = pb_pool.tile([128, 4, 128], f32, tag="pb",
                                  name=f"pb{h}_{g}")
                for q in range(8):
                    nc.tensor.matmul(
                        out=pb[:, :, :],
                        lhsT=r2[:, q, :], rhs=ptgt[:, :, q, :],
                        start=(q == 0), stop=(q == 7))
                return pb

            def emit_exp(h, g, pb):
                nc.scalar.activation(
                    out=eeT[h][:, 4 * g:4 * g + 4, :], in_=pb[:, :, :],
                    func=mybir.ActivationFunctionType.Exp, scale=EXPSCALE)

            def emit_zpart(h, zg):
                """Partial softmax denom over j in [8*zg, 8*zg+8). Pool is
                idle in sweep 1, so h=0 partials run there."""
                eng = nc.gpsimd if h == 0 else nc.vector
                with nc.allow_low_precision("softmax denom in bf16"):
                    ee = eeT[h][:, 8 * zg:8 * zg + 8, :]
                    t4 = work.tile([128, 4, B], bf16, tag=f"zt4_{h}_{zg}")
                    eng.tensor_tensor(
                        out=t4[:, :, :], in0=ee[:, 0:4, :], in1=ee[:, 4:8, :],
                        op=mybir.AluOpType.add)
                    t2 = work.tile([128, 2, B], bf16, tag=f"zt2_{h}_{zg}")
                    eng.tensor_tensor(
                        out=t2[:, :, :], in0=t4[:, 0:2, :], in1=t4[:, 2:4, :],
                        op=mybir.AluOpType.add)
                    z1 = work.tile([128, B], bf16, tag=f"zp{h}_{zg}")
                    eng.tensor_tensor(
                        out=z1[:, :], in0=t2[:, 0, :], in1=t2[:, 1, :],
                        op=mybir.AluOpType.add)
                    zp[h][zg] = z1

            def emit_zfinal_x2t(h):
                with nc.allow_low_precision("softmax denom in bf16"):
                    za = work.tile([128, B], bf16, tag=f"za{h}")
                    nc.vector.tensor_tensor(
                        out=za[:, :], in0=zp[h][0][:, :], in1=zp[h][1][:, :],
                        op=mybir.AluOpType.add)
                    zb = work.tile([128, B], bf16, tag=f"zb{h}")
                    nc.vector.tensor_tensor(
                        out=zb[:, :], in0=zp[h][2][:, :], in1=zp[h][3][:, :],
                        op=mybir.AluOpType.add)
                    zs = work.tile([128, B], bf16, tag=f"zs{h}")
                    nc.vector.tensor_tensor(
                        out=zs[:, :], in0=za[:, :], in1=zb[:, :],
                        op=mybir.AluOpType.add)
                    rzh = consts.tile([128, B], bf16, tag=f"rz{h}",
                                      name=f"rz{h}")
                    nc.vector.reciprocal(out=rzh[:, :], in_=zs[:, :])
                    rz[h] = rzh
                nc.vector.tensor_tensor(
                    out=_strided(x2t[:, h, 0:1], [[256, 8], [1, B]]),
                    in0=_strided(xt[:, h, 0:1], [[256, 8], [1, B]]),
                    in1=_bc_mid(rz[h][:, :], 8), op=mybir.AluOpType.mult)

            def emit_B(kt, tail=False):
                h = kt % 2
                yt = ypool.tile([128, J, B], bf16, tag="yt", name=f"yt{kt}")
                bs = TAIL_BSPLIT if tail else YT_BSPLIT
                for jq in range(4):
                    o = yt[:, 8 * jq:8 * jq + 8, :]
                    i0 = _bc_mid(x2t[:, kt, :], 8)
                    i1 = eeT[h][:, 8 * jq:8 * jq + 8, :]
                    nc.vector.tensor_tensor(
                        out=o[:, :, 0:bs], in0=i0[:, :, 0:bs],
                        in1=i1[:, :, 0:bs], op=mybir.AluOpType.mult)
                    nc.gpsimd.tensor_tensor(
                        out=o[:, :, bs:B], in0=i0[:, :, bs:B],
                        in1=i1[:, :, bs:B], op=mybir.AluOpType.mult)
                # ps2 is one accumulation group (multiple start=True groups
                # on one PSUM bank reset the accumulation window on HW)
                for j in range(J):
                    nc.tensor.matmul(
                        out=ps2[:, 16 * j:16 * j + 16],
                        lhsT=yt[:, j, :], rhs=ws[:, j, kt, :],
                        start=(kt == 0 and j == 0),
                        stop=(kt == 15 and j == J - 1))

            # ---------- pass 2: software-pipelined chunk loop ----------
            # sweep h=0, then h=1 overlapped with even-kt yT/s2; odd kt after.
            def after_exp(hh, gg):
                if gg % 2 == 1:
                    emit_zpart(hh, (gg - 1) // 2)
                if hh == 0 and gg == 7:
                    emit_zfinal_x2t(0)
                if hh == 1:                 # interleave even-kt yT/s2
                    emit_B(2 * gg)

            sched = [(0, j) for j in range(J)] + [(1, j) for j in range(J)]
            ptgt_cur = [None]
            todo_R = []   # (ready_ci, h, g, ptgt): R deferred 1 chunk
            emit_v1T(0)
            pa_next = emit_AT(*sched[0])
            for ci in range(len(sched) + 2):
                if ci < len(sched):
                    h, j = sched[ci]
                    pa = pa_next
                    if ci + 1 < len(sched):
                        hn, jn = sched[ci + 1]
                        if hn == 0 and jn % 8 == 0 and jn > 0:
                            emit_v1T(jn // 8)
                        pa_next = emit_AT(hn, jn)
                    if j % 4 == 0:
                        ptgt_cur[0] = ptgpool.tile(
                            [128, 4, 8, 128], bf16, tag="ptg",
                            name=f"ptg{ci // 4}")
                    emit_mult(h, j, pa, ptgt_cur[0])
                if todo_R and todo_R[0][0] <= ci:
                    _, hh, gg, pt_ = todo_R.pop(0)
                    emit_exp(hh, gg, emit_R(hh, gg, pt_))
                    after_exp(hh, gg)
                if ci < len(sched) and sched[ci][1] % 4 == 3:
                    todo_R.append((ci + 1, sched[ci][0], sched[ci][1] // 4,
                                   ptgt_cur[0]))
            emit_zfinal_x2t(1)
            # dummy sqrt: hoists the Sqrt act-table load into the idle ACT
            # window during the tail, off the drain critical path (DMA'd to
            # scratch so dead-code elimination keeps it)
            dum = work.tile([B, 1], f32, tag="dumsq")
            nc.scalar.sqrt(out=dum[:, :], in_=eeT[1][:, 0, 0:1])
            nc.scalar.dma_start(out=dscr_o.ap(), in_=dum[:, :])
            for kt in range(1, 16, 2):
                emit_B(kt, tail=True)

            # ---------- s2 drain + squash per 128-col block ----------
            s2c = work.tile([B, JD], f32, tag="s2c")
            s2 = consts.tile([B, JD], f32, tag="s2")
            v2 = consts.tile([B, JD], f32, tag="v2")
            # ps2 deps are tile-granular, so one full-width copy beats four
            nc.scalar.copy(out=s2c[:, :], in_=ps2[:, :])
            for t in range(4):
                eng = [nc.sync, nc.scalar, nc.sync, nc.scalar][t]
                eng.dma_start(out=cc2i[t].ap(),
                              in_=s2c[:, 128 * t:128 * t + 128])
                allreduce(cc2i[t], cc2o[t], eng=eng)
                eng.dma_start(out=s2[:, 128 * t:128 * t + 128],
                              in_=cc2o[t].ap())
            # single full-width squash once all 4 blocks are in: fewer
            # serialized small DVE ops than a per-block cascade
            _squash_full(nc, work, s2, v2)
            nc.sync.dma_start(out=yout.ap(), in_=v2[:, :])

    nc.compile()
    return nc


_NC_CACHE = None


def _get_nc():
    global _NC_CACHE
    if _NC_CACHE is None:
        _NC_CACHE = build_nc()
    return _NC_CACHE


def _prep_inputs(x, W):
    """Pure layout transform + bf16 cast: returns per-core input dicts."""
    bf = ml_dtypes.bfloat16
    x = np.asarray(x, dtype=np.float32)
    W = np.asarray(W, dtype=np.float32)
    xc = x.reshape(B, NC_, IS, E)             # [b, c, il, e]
    Wc = W[0].reshape(NC_, IS, J, D, E)       # [c, il, j, d, e]
    ident = np.eye(128, dtype=np.float32)
    # r2[(i16*8+e), q, i'] = 1 iff i' == 16q+i16  (e-reduce matrix)
    r2 = np.zeros((128, 8, 128), dtype=np.float32)
    i16 = np.arange(16)
    for q in range(8):
        for e in range(E):
            r2[i16 * 8 + e, q, 16 * q + i16] = 1.0
    outs = []
    for c in range(NC_):
        xv = xc[:, c]                          # [B, 256, 8]
        Wv = Wc[c]                             # [256, J, D, E]
        # xt[lane, kt=(e,half), b] = x[b, half*128+lane, e]
        a = xv.transpose(1, 2, 0).reshape(2, 128, E, B)   # [half, lane, e, b]
        xt = np.ascontiguousarray(a.transpose(1, 2, 0, 3)).reshape(128, 16, B)
        # xt2[(i16*8+e), (h*8+q), b] = x[b, 128h+16q+i16, e]
        a = xv.reshape(B, 2, 8, 16, E)                    # [b, h, q, i16, e]
        xt2 = np.ascontiguousarray(
            a.transpose(3, 4, 1, 2, 0)).reshape(128, 16, B)
        # w1[lane, t, kt, col] = W[il(kt,lane), j, d, e(kt)]/32, (j,d)=128t+col
        a = Wv.reshape(2, 128, J, D, E).transpose(1, 4, 0, 2, 3)
        w1 = np.ascontiguousarray(a).reshape(128, 16, JD) * W1SCALE
        w1 = np.ascontiguousarray(
            w1.reshape(128, 16, 4, 128).transpose(0, 2, 1, 3))
        # wt[32*(j%4)+d, j//4, h, q, (i16*8+e)] = W[128h+16q+i16, j, d, e]
        # (rows 16..31 of each 32-slot are zero: K=32 matmuls, 32-aligned)
        a = Wv.reshape(2, 8, 16, J, D, E)                 # [h, q, i16, j, d, e]
        tmp = a.transpose(4, 3, 0, 1, 2, 5)               # [d, j, h, q, i16, e]
        arr = np.zeros((32, J, 2, 8, 128), dtype=np.float32)
        arr[:16] = tmp.reshape(16, J, 2, 8, 128) * W8SCALE
        wt = np.ascontiguousarray(
            arr.reshape(32, 8, 4, 2, 8, 128).transpose(2, 0, 1, 3, 4, 5)
        ).reshape(128, 8, 2, 8, 128)
        # ws[lane, j, kt, d] = W[il(kt,lane), j, d, e(kt)]
        a = Wv.reshape(2, 128, J, D, E).transpose(1, 2, 4, 0, 3)
        ws = np.ascontiguousarray(a).reshape(128, J, 16, D)
        f8m = ml_dtypes.float8_e4m3fn if USE_F8 else bf
        outs.append({
            "xt": xt.astype(bf), "xt2": xt2.astype(bf),
            "xp8": xt.astype(f8m), "w1": w1.astype(f8m),
            "wt": wt.astype(f8m), "ws": ws.astype(bf), "r2": r2.astype(bf),
            "identf": ident,
        })
    return outs


def run(x, W, trace=False, **kw):
    in_maps = _prep_inputs(x, W)
    nc = _get_nc()
    res = run_bass_kernel_spmd(nc, in_maps, core_ids=list(range(NC_)),
                               trace=trace, **kw)
    out = res.results[0]["yout"].reshape(B, J, D).astype(np.float32)
    return out, res


def kernel(x, W):
    out, _ = run(x, W)
    return out


# revision 84
# speedup vs baseline: 1.0234x; 1.0234x over previous
"""DigitCaps dynamic-routing kernel for 8 Trainium2 NeuronCores — v5.

Math (reference):
  u_hat[b,i,j,d] = sum_e W[0,i,j,d,e] * x[b,i,e]
  2 routing iterations; iteration 1 has b=0 so c = 1/32 exactly:
    s1 = (1/32) sum_i u_hat ;  v1 = squash(s1)
    b2 = sum_d u_hat * v1    ;  c2 = softmax_j(b2)
    s2 = sum_i c2 * u_hat    ;  v2 = squash(s2)   -> output

Structure (per core, i sharded 8 ways, batch b=128):
  s1: K=2048 GEMM in 4 column blocks; AllReduce + squash + v1-transpose
      pipelined per 128-col block so pass 2a starts early.
  pass 2 runs in TRANSPOSED land, u_hat never materialized:
    A_T[(i16,e8),q,b] = sum_d W[i,j,d,e] v1[b,j,d]    (8 K=16 GEMMs/chunk)
    P_T = A_T * xT                                    (one mult/chunk)
    b2T[i,b] = sum_{(i16,e8)} R_q * P_T               (8 K=128 GEMMs per
                                                       4-chunk group, N=512,
                                                       PSUM-accumulated; R_q is
                                                       a constant 0/1 matrix —
                                                       the e-reduce runs on PE)
    eeT = exp(b2T)    (ACT exp on PSUM->SBUF copy-out, batched 4 j's)
    zT,rzT = softmax denom (incremental partials per 8 j's), x2t = xT*rzT
    yT[(i),j,b] = x2t * eeT                           (broadcast mult)
    s2[b,(j,d)] = sum_{kt=(e,h)} sum_i yT * W         (per-(j,kt) GEMM, N=16)
  s2 drains in 4 column blocks (copy+AllReduce+squash+out per block).
  Tail (odd-kt yT after rz[1]) splits each mult DVE/Pool by batch columns.
"""

import sys
for _p in ("/opt/pypackages", "/opt/trn_rl_repo"):
    if _p not in sys.path:
        sys.path.insert(0, _p)

import numpy as np
import ml_dtypes

import concourse.bass as bass
import concourse.bacc as bacc
import concourse.tile as tile
from concourse import mybir
from concourse.bass_utils import run_bass_kernel_spmd

B = 128
I = 2048
E = 8
J = 32
D = 16
JD = J * D          # 512
NC_ = 8             # cores
IS = I // NC_       # 256 in_caps per core
EPS = 1e-8

f32 = mybir.dt.float32
bf16 = mybir.dt.bfloat16
USE_F8 = True    # fp8 for the routing-path weights (w1/wt/xp8/v1t2)
f8 = mybir.dt.float8e4 if USE_F8 else bf16
# fp8e4m3 min-normal is 2^-6: W (~0.01) and v1 (~0.01..0.2) must be scaled
# into the normal range, compensated at the s1c copy and the exp
W8SCALE = 64.0 if USE_F8 else 1.0         # host multiplier on wt
W1SCALE = 64.0 if USE_F8 else (1.0 / J)   # host multiplier on w1
S1CSCALE = 1.0 / (J * 64.0) if USE_F8 else 1.0
V1TSCALE = 16.0 if USE_F8 else 1.0        # ACT scale on the v1t2 copy
EXPSCALE = 1.0 / (64.0 * 16.0) if USE_F8 else 1.0

# ---- engine assignment knobs (tuned against TimelineSim) ----
# 64 chunks (chunk = h*32 + j): who moves A_T from PSUM and multiplies by x:
#  'act'  = ACT copies PSUM->bf16 SBUF, DVE does the 2x mult
#  'dve'  = DVE mult direct from PSUM (1x)
#  'pool' = Pool mult direct from PSUM
def _ppath(c):
    # No 'pool' mults: Pool's 2127ns latency in the R dependency chain
    # costs more than its throughput adds (Pool still runs yT b-splits).
    if c < 32:   # sweep 1
        if c < 12:
            return 'dve' if c % 2 == 1 else 'act'
        return ('act', 'dve', 'dve', 'act')[c % 4]
    if c >= 62:  # end-game: skip the 2-stage ACT path on the last chunks
        return 'dve'
    return 'act'  # sweep 2
P_PATH = [_ppath(c) for c in range(64)]
# yT mults: every quarter-unit is split DVE/Pool by batch columns
YT_BSPLIT = 88      # sweep-2 even kt: DVE b[0:88], Pool b[88:128]
TAIL_BSPLIT = 104   # tail odd kt: DVE b[0:104], Pool b[104:128]
R_LAG = 3           # chunks between a group's last mult and its R batch
EXP_LAG = 1         # extra chunks between R and the exp copy-out


def _bc(ap, n):
    """Broadcast an AP along a new innermost dim of size n (step 0)."""
    return bass.AP(tensor=ap.tensor, offset=ap.offset, ap=[*ap.ap, [0, n]])


def _bc_mid(ap, n):
    """Broadcast an AP along a new dim of size n inserted before the last
    free dim (step 0)."""
    return bass.AP(tensor=ap.tensor, offset=ap.offset,
                   ap=[*ap.ap[:-1], [0, n], ap.ap[-1]])


def _strided(ap, dims):
    """Replace the free dims of a [P, 1]-sliced AP with custom [step, num]
    pairs (partition dim kept)."""
    return bass.AP(tensor=ap.tensor, offset=ap.offset, ap=[ap.ap[0], *dims])


def _squash_full(nc, pool, s_sb, v_sb):
    """v = |s|^2/(1+|s|^2) * s/(|s|+eps) over all 32 j at once."""
    sq = pool.tile([B, JD], f32, tag="fsqs")
    nc.vector.tensor_mul(sq[:, :], s_sb[:, :], s_sb[:, :])
    n2 = pool.tile([B, J], f32, tag="fsqn2")
    nc.vector.tensor_reduce(
        out=n2[:, :], in_=sq[:, :].rearrange("p (j d) -> p j d", d=D),
        axis=mybir.AxisListType.X, op=mybir.AluOpType.add)
    nrm = pool.tile([B, J], f32, tag="fsqn")
    nc.scalar.sqrt(out=nrm[:, :], in_=n2[:, :])
    a1 = pool.tile([B, J], f32, tag="fsqa")
    nc.vector.tensor_scalar_add(a1[:, :], n2[:, :], 1.0)
    b1 = pool.tile([B, J], f32, tag="fsqb")
    nc.vector.tensor_scalar_add(b1[:, :], nrm[:, :], EPS)
    den = pool.tile([B, J], f32, tag="fsqd")
    nc.vector.tensor_mul(den[:, :], a1[:, :], b1[:, :])
    rden = pool.tile([B, J], f32, tag="fsqr")
    nc.vector.reciprocal(out=rden[:, :], in_=den[:, :])
    sc = pool.tile([B, J], f32, tag="fsqc")
    nc.vector.tensor_mul(sc[:, :], n2[:, :], rden[:, :])
    nc.vector.tensor_tensor(
        out=v_sb[:, :].rearrange("p (j d) -> p j d", d=D),
        in0=s_sb[:, :].rearrange("p (j d) -> p j d", d=D),
        in1=_bc(sc[:, :], D), op=mybir.AluOpType.mult)


def _squash_blk(nc, pool, s_sb, v_sb, t, tag):
    """v = |s|^2/(1+|s|^2) * s/(|s|+eps) for j-block t (8 j's, 128 cols)."""
    nj = J // 4
    c0 = 128 * t
    sv = s_sb[:, c0:c0 + 128]
    sq = pool.tile([B, 128], f32, tag=f"sqs{tag}", name=f"sqs{tag}{t}")
    nc.vector.tensor_mul(sq[:, :], sv, sv)
    n2 = pool.tile([B, nj], f32, tag=f"sqn2{tag}", name=f"sqn2{tag}{t}")
    nc.vector.tensor_reduce(
        out=n2[:, :], in_=sq[:, :].rearrange("p (j d) -> p j d", d=D),
        axis=mybir.AxisListType.X, op=mybir.AluOpType.add)
    nrm = pool.tile([B, nj], f32, tag=f"sqn{tag}", name=f"sqn{tag}{t}")
    nc.scalar.sqrt(out=nrm[:, :], in_=n2[:, :])
    a1 = pool.tile([B, nj], f32, tag=f"sqa{tag}", name=f"sqa{tag}{t}")
    nc.vector.tensor_scalar_add(a1[:, :], n2[:, :], 1.0)
    b1 = pool.tile([B, nj], f32, tag=f"sqb{tag}", name=f"sqb{tag}{t}")
    nc.vector.tensor_scalar_add(b1[:, :], nrm[:, :], EPS)
    den = pool.tile([B, nj], f32, tag=f"sqd{tag}", name=f"sqd{tag}{t}")
    nc.vector.tensor_mul(den[:, :], a1[:, :], b1[:, :])
    rden = pool.tile([B, nj], f32, tag=f"sqr{tag}", name=f"sqr{tag}{t}")
    nc.vector.reciprocal(out=rden[:, :], in_=den[:, :])
    sc = pool.tile([B, nj], f32, tag=f"sqc{tag}", name=f"sqc{tag}{t}")
    nc.vector.tensor_mul(sc[:, :], n2[:, :], rden[:, :])
    nc.vector.tensor_tensor(
        out=v_sb[:, c0:c0 + 128].rearrange("p (j d) -> p j d", d=D),
        in0=sv.rearrange("p (j d) -> p j d", d=D),
        in1=_bc(sc[:, :], D), op=mybir.AluOpType.mult)


def build_nc(num_devices=NC_, with_cc=True):
    nc = bacc.Bacc("TRN2", target_bir_lowering=False, debug=False,
                   num_devices=num_devices)
    # host-prepped per-core inputs (see _prep_inputs for layouts)
    # w1/wt/xp8 feed only the routing-coefficient path (b2 is ~7e-4, so
    # exp(b2)~1+b2 and softmax damps any error there by ~1e3x): fp8 is safe
    xt_d = nc.dram_tensor("xt", [128, 16, B], bf16, kind="ExternalInput")
    xt2_d = nc.dram_tensor("xt2", [128, 16, B], bf16, kind="ExternalInput")
    xp8_d = nc.dram_tensor("xp8", [128, 16, B], f8, kind="ExternalInput")
    w1_d = nc.dram_tensor("w1", [128, 4, 16, 128], f8, kind="ExternalInput")
    # wt rows = 32*(j%4) + d (d<16 real, 16..31 zero-padded): K=32 matmuls
    # with lhsT/rhs/tile_position all 32-aligned (Ldweights requires it)
    wt_d = nc.dram_tensor("wt", [128, 8, 2, 8, 128], f8, kind="ExternalInput")
    ws_d = nc.dram_tensor("ws", [128, J, 16, D], bf16, kind="ExternalInput")
    r2_d = nc.dram_tensor("r2", [128, 8, 128], bf16, kind="ExternalInput")
    idf_d = nc.dram_tensor("identf", [128, 128], f32, kind="ExternalInput")
    yout = nc.dram_tensor("yout", [B, JD], f32, kind="ExternalOutput")
    dscr_o = nc.dram_tensor("dscr", [B, 1], f32, kind="ExternalOutput")

    cc1i = [nc.dram_tensor(f"cc1i{t}", [B, 128], f32) for t in range(4)]
    cc1o = [nc.dram_tensor(f"cc1o{t}", [B, 128], f32, addr_space="Shared")
            for t in range(4)]
    cc2i = [nc.dram_tensor(f"cc2i{t}", [B, 128], f32) for t in range(4)]
    cc2o = [nc.dram_tensor(f"cc2o{t}", [B, 128], f32, addr_space="Shared")
            for t in range(4)]
    rgroups = [list(range(num_devices))]

    def allreduce(cin, cout, eng=None):
        if with_cc:
            nc.gpsimd.collective_compute(
                "AllReduce", mybir.AluOpType.add, replica_groups=rgroups,
                ins=[cin.ap()], outs=[cout.ap()])
        else:
            (eng or nc.sync).dma_start(out=cout.ap(), in_=cin.ap())

    with tile.TileContext(nc) as tc:
        with (
            tc.tile_pool(name="consts", bufs=1) as consts,
            tc.tile_pool(name="work", bufs=1) as work,
            tc.tile_pool(name="latep", bufs=1) as latep,
            tc.tile_pool(name="ptg", bufs=3) as ptgpool,
            tc.tile_pool(name="pacp", bufs=5) as pacp,
            tc.tile_pool(name="ypool", bufs=2) as ypool,
            tc.tile_pool(name="pa", bufs=3, space="PSUM") as pa_pool,
            tc.tile_pool(name="pb", bufs=1, space="PSUM") as pb_pool,
            tc.tile_pool(name="pacc", bufs=1, space="PSUM") as pacc,
        ):
            # ---------- input DMAs (ordered by first use) ----------
            gd_xt2 = latep.tile([128, 16, B], bf16, tag="lxt2",
                                name="gd_xt2")
            gd_r2 = latep.tile([128, 8, 128], bf16, tag="lr2", name="gd_r2")
            gd_wt = latep.tile([128, 7, 2, 8, 128], f8, tag="lwt",
                               name="gd_wt")
            gd_ws = latep.tile([128, J, 16, D], bf16, tag="lws",
                               name="gd_ws")
            nc.vector.memset(gd_xt2[:, 0:1, 0:1], 0.0)
            nc.vector.memset(gd_r2[:, 0:1, 0:1], 0.0)
            nc.vector.memset(gd_wt[:, 0:1, 0:1, 0:1, 0:1], 0.0)
            nc.vector.memset(gd_ws[:, 0:1, 0:1, 0:1], 0.0)
            identf = consts.tile([128, 128], f32, tag="identf")
            nc.sync.dma_start(out=identf[:, :], in_=idf_d.ap())
            xt = consts.tile([128, 16, B], bf16, tag="xt")
            nc.sync.dma_start(out=xt[:, :, :], in_=xt_d.ap())
            xp8 = consts.tile([128, 16, B], f8, tag="xp8")
            nc.sync.dma_start(out=xp8[:, :, :], in_=xp8_d.ap())
            w1 = consts.tile([128, 4, 16, 128], f8, tag="w1")
            for t in range(4):   # column blocks to pipeline pass 1
                nc.sync.dma_start(out=w1[:, t, :, :],
                                  in_=w1_d.ap()[:, t, :, :])
            wt0 = consts.tile([128, 1, 2, 8, 128], f8, tag="wt0")
            nc.sync.dma_start(out=wt0[:, 0, :, :, :],
                              in_=wt_d.ap()[:, 0, :, :, :])

            # ---------- pass 1 + AR1 + squash + v1T, per 128-col block ----
            # each 128-col block accumulates in its OWN psum tile (from the
            # pa ring) because PSUM dependencies are tile-granular: a shared
            # [B,512] tile would stall every copy until the last matmul
            ps1b = []
            s1c = work.tile([B, JD], f32, tag="s1c")
            s1 = consts.tile([B, JD], f32, tag="s1")
            v1 = consts.tile([B, JD], f32, tag="v1")
            # v1t2[32*(j%4)+d, j//4, b] = v1[b, 16j+d] (rows 16..31 zero)
            v1t2 = consts.tile([128, 8, B], f8, tag="v1t2")
            v1pad = work.tile([B, 8, 128], f32, tag="v1pad")
            nc.vector.memset(v1pad[:, :, :], 0.0)
            for t in range(4):
                pt1 = pa_pool.tile([B, 8, 128], f32, tag="pa",
                                   name=f"ps1b{t}")
                ps1b.append(pt1)
                for kt in range(16):
                    nc.tensor.matmul(
                        out=pt1[:, 0, :],
                        lhsT=xp8[:, kt, :],
                        rhs=w1[:, t, kt, :],
                        start=(kt == 0), stop=(kt == 15))
                nc.scalar.activation(
                    out=s1c[:, 128 * t:128 * t + 128], in_=pt1[:, 0, :],
                    func=mybir.ActivationFunctionType.Copy, scale=S1CSCALE)
            # AR chains ride the two HWDGE queues (SP even blocks, ACT odd)
            # so they dispatch in parallel and their small DMAs claim the
            # DMA engines before the late input loads
            for t in range(4):
                eng = [nc.sync, nc.scalar, nc.sync, nc.scalar][t]
                eng.dma_start(out=cc1i[t].ap(),
                              in_=s1c[:, 128 * t:128 * t + 128])
                allreduce(cc1i[t], cc1o[t], eng=eng)
                eng.dma_start(out=s1[:, 128 * t:128 * t + 128],
                              in_=cc1o[t].ap())
                _squash_blk(nc, work, s1, v1, t, "v1")
            # late inputs are WAR-gated: their tiles reuse pool buffers
            # whose previous tile is read by an s1-dependent op, so the DMAs
            # carry a real semaphore and the DMA-device FIFO (ordered by
            # request time) serves the AR-chain hops first
            gr = work.tile([B, 1], f32, tag="gater")
            nc.vector.tensor_tensor(out=gr[:, :], in0=gd_xt2[:, 0:1, 0],
                                    in1=s1[:, 128:129],
                                    op=mybir.AluOpType.add)
            nc.vector.tensor_tensor(out=gr[:, :], in0=gd_wt[:, 0:1, 0, 0, 0],
                                    in1=s1[:, 128:129],
                                    op=mybir.AluOpType.add)
            nc.vector.tensor_tensor(out=gr[:, :], in0=gd_ws[:, 0:1, 0, 0],
                                    in1=s1[:, 128:129],
                                    op=mybir.AluOpType.add)
            nc.vector.tensor_tensor(out=gr[:, :], in0=gd_r2[:, 0:1, 0],
                                    in1=s1[:, 128:129],
                                    op=mybir.AluOpType.add)
            xt2 = latep.tile([128, 16, B], bf16, tag="lxt2")
            nc.gpsimd.dma_start(out=xt2[:, :, :], in_=xt2_d.ap())
            r2 = latep.tile([128, 8, 128], bf16, tag="lr2")
            nc.gpsimd.dma_start(out=r2[:, :, :], in_=r2_d.ap())
            wt = latep.tile([128, 7, 2, 8, 128], f8, tag="lwt")
            for g in range(7):
                nc.gpsimd.dma_start(out=wt[:, g, :, :, :],
                                    in_=wt_d.ap()[:, g + 1, :, :, :])
            ws = latep.tile([128, J, 16, D], bf16, tag="lws")
            for g in range(4):
                nc.gpsimd.dma_start(
                    out=ws[:, 8 * g:8 * g + 8, :, :],
                    in_=ws_d.ap()[:, 8 * g:8 * g + 8, :, :])

            def emit_v1T(t):
                """Transpose v1 block t (j in [8t, 8t+8)) into padded 32-row
                slots of v1t2 (deferred into the chunk loop so it doesn't
                head-of-line block the PE queue)."""
                # v1pad[b, tt, 32s+d] = v1[b, 64tt+16s+d] for the 2 tt's
                for s in range(4):
                    nc.vector.tensor_copy(
                        _strided(v1pad[:, 2 * t, 32 * s:32 * s + 1],
                                 [[128, 2], [1, 16]]),
                        _strided(v1[:, 128 * t + 16 * s:128 * t + 16 * s + 1],
                                 [[64, 2], [1, 16]]))
                tpv = pacc.tile([128, 4, 128], f32, tag="acc",
                                name=f"tpv{t}")
                for k in range(2):
                    nc.tensor.transpose(
                        out=tpv[:, k, :],
                        in_=v1pad[:, 2 * t + k, :], identity=identf[:, :])
                nc.scalar.activation(
                    out=v1t2[:, 2 * t:2 * t + 2, :], in_=tpv[:, 0:2, :],
                    func=mybir.ActivationFunctionType.Copy, scale=V1TSCALE)

            # ---------- pass 2a state ----------
            eeT = [consts.tile([128, J, B], bf16, tag=f"eeT{h}",
                               name=f"eeT{h}") for h in range(2)]
            zp = [[None] * 4, [None] * 4]   # partial z per 8-j group
            rz = [None, None]
            x2t = consts.tile([128, 16, B], bf16, tag="x2t")
            ps2 = pacc.tile([B, JD], f32, tag="acc", name="ps2")

            def emit_AT(h, j):
                """8 K=32 GEMMs (16 zero-pad rows): pa[:, q, :] =
                W_hjq^T . v1_j^T; returns pa."""
                pa = pa_pool.tile([B, 8, 128], f32, tag="pa",
                                  name=f"pa{h}_{j}")
                s = j % 4
                g = j // 4
                wsrc = wt0[32 * s:32 * s + 32, 0, h, :, :] if g == 0 else \
                    wt[32 * s:32 * s + 32, g - 1, h, :, :]
                for q in range(8):
                    nc.tensor.matmul(
                        out=pa[:, q, :],
                        lhsT=wsrc[:, q, :],
                        rhs=v1t2[32 * s:32 * s + 32, j // 4, :],
                        start=True, stop=True,
                        tile_position=(32 * s, 0))
                return pa

            def emit_mult(h, j, pa, ptgt):
                """P_T mult into ptgt[:, j%4, :, :] via the chunk's engine."""
                chunk = h * J + j
                xs = xt2[:, 8 * h:8 * h + 8, :]
                out = ptgt[:, j % 4, :, :]
                path = P_PATH[chunk]
                if path == 'act':
                    pac = pacp.tile([128, 8, 128], bf16, tag="pac",
                                    name=f"pac{chunk}")
                    nc.scalar.copy(out=pac[:, :, :], in_=pa[:, :, :])
                    nc.vector.tensor_tensor(
                        out=out, in0=pac[:, :, :], in1=xs,
                        op=mybir.AluOpType.mult)
                elif path == 'pcopy':
                    pac = pacp.tile([128, 8, 128], bf16, tag="pac",
                                    name=f"pac{chunk}")
                    nc.gpsimd.tensor_copy(pac[:, :, :], pa[:, :, :])
                    nc.vector.tensor_tensor(
                        out=out, in0=pac[:, :, :], in1=xs,
                        op=mybir.AluOpType.mult)
                elif path == 'dve':
                    nc.vector.tensor_tensor(
                        out=out, in0=pa[:, :, :], in1=xs,
                        op=mybir.AluOpType.mult)
                else:
                    nc.gpsimd.tensor_tensor(
                        out=out, in0=pa[:, :, :], in1=xs,
                        op=mybir.AluOpType.mult)

            def emit_R(h, g, ptgt):
                """Batched e-reduce for 4 chunks; returns the psum tile."""
                pb = pb_pool.tile([128, 4, 128], f32, tag="pb",
                                  name=f"pb{h}_{g}")
                for q in range(8):
                    nc.tensor.matmul(
                        out=pb[:, :, :],
                        lhsT=r2[:, q, :], rhs=ptgt[:, :, q, :],
                        start=(q == 0), stop=(q == 7))
                return pb

            def emit_exp(h, g, pb):
                nc.scalar.activation(
                    out=eeT[h][:, 4 * g:4 * g + 4, :], in_=pb[:, :, :],
                    func=mybir.ActivationFunctionType.Exp, scale=EXPSCALE)

            def emit_zpart(h, zg):
                """Partial softmax denom over j in [8*zg, 8*zg+8). Pool is
                idle in sweep 1, so h=0 partials run there."""
                eng = nc.gpsimd if h == 0 else nc.vector
                with nc.allow_low_precision("softmax denom in bf16"):
                    ee = eeT[h][:, 8 * zg:8 * zg + 8, :]
                    t4 = work.tile([128, 4, B], bf16, tag=f"zt4_{h}_{zg}")
                    eng.tensor_tensor(
                        out=t4[:, :, :], in0=ee[:, 0:4, :], in1=ee[:, 4:8, :],
                        op=mybir.AluOpType.add)
                    t2 = work.tile([128, 2, B], bf16, tag=f"zt2_{h}_{zg}")
                    eng.tensor_tensor(
                        out=t2[:, :, :], in0=t4[:, 0:2, :], in1=t4[:, 2:4, :],
                        op=mybir.AluOpType.add)
                    z1 = work.tile([128, B], bf16, tag=f"zp{h}_{zg}")
                    eng.tensor_tensor(
                        out=z1[:, :], in0=t2[:, 0, :], in1=t2[:, 1, :],
                        op=mybir.AluOpType.add)
                    zp[h][zg] = z1

            def emit_zfinal_x2t(h):
                with nc.allow_low_precision("softmax denom in bf16"):
                    za = work.tile([128, B], bf16, tag=f"za{h}")
                    nc.vector.tensor_tensor(
                        out=za[:, :], in0=zp[h][0][:, :], in1=zp[h][1][:, :],
                        op=mybir.AluOpType.add)
                    zb = work.tile([128, B], bf16, tag=f"zb{h}")
                    nc.vector.tensor_tensor(
                        out=zb[:, :], in0=zp[h][2][:, :], in1=zp[h][3][:, :],
                        op=mybir.AluOpType.add)
                    zs = work.tile([128, B], bf16, tag=f"zs{h}")
                    nc.vector.tensor_tensor(
                        out=zs[:, :], in0=za[:, :], in1=zb[:, :],
                        op=mybir.AluOpType.add)
                    rzh = consts.tile([128, B], bf16, tag=f"rz{h}",
                                      name=f"rz{h}")
                    nc.vector.reciprocal(out=rzh[:, :], in_=zs[:, :])
                    rz[h] = rzh
                nc.vector.tensor_tensor(
                    out=_strided(x2t[:, h, 0:1], [[256, 8], [1, B]]),
                    in0=_strided(xt[:, h, 0:1], [[256, 8], [1, B]]),
                    in1=_bc_mid(rz[h][:, :], 8), op=mybir.AluOpType.mult)

            def emit_B(kt, tail=False):
                h = kt % 2
                yt = ypool.tile([128, J, B], bf16, tag="yt", name=f"yt{kt}")
                bs = TAIL_BSPLIT if tail else YT_BSPLIT
                for jq in range(4):
                    o = yt[:, 8 * jq:8 * jq + 8, :]
                    i0 = _bc_mid(x2t[:, kt, :], 8)
                    i1 = eeT[h][:, 8 * jq:8 * jq + 8, :]
                    nc.vector.tensor_tensor(
                        out=o[:, :, 0:bs], in0=i0[:, :, 0:bs],
                        in1=i1[:, :, 0:bs], op=mybir.AluOpType.mult)
                    nc.gpsimd.tensor_tensor(
                        out=o[:, :, bs:B], in0=i0[:, :, bs:B],
                        in1=i1[:, :, bs:B], op=mybir.AluOpType.mult)
                # ps2 is one accumulation group (multiple start=True groups
                # on one PSUM bank reset the accumulation window on HW)
                for j in range(J):
                    nc.tensor.matmul(
                        out=ps2[:, 16 * j:16 * j + 16],
                        lhsT=yt[:, j, :], rhs=ws[:, j, kt, :],
                        start=(kt == 0 and j == 0),
                        stop=(kt == 15 and j == J - 1))

            # ---------- pass 2: software-pipelined chunk loop ----------
            # sweep h=0, then h=1 overlapped with even-kt yT/s2; odd kt after.
            def after_exp(hh, gg):
                if gg % 2 == 1:
                    emit_zpart(hh, (gg - 1) // 2)
                if hh == 0 and gg == 7:
                    emit_zfinal_x2t(0)
                if hh == 1:                 # interleave even-kt yT/s2
                    emit_B(2 * gg)

            sched = [(0, j) for j in range(J)] + [(1, j) for j in range(J)]
            ptgt_cur = [None]
            todo_R = []   # (ready_ci, h, g, ptgt): R deferred R_LAG chunks
            todo_E = []   # (ready_ci, h, g, pb): exp deferred EXP_LAG more
            emit_v1T(0)
            pa_next = emit_AT(*sched[0])
            for ci in range(len(sched) + 2 + R_LAG + EXP_LAG):
                if ci < len(sched):
                    h, j = sched[ci]
                    pa = pa_next
                    if ci + 1 < len(sched):
                        hn, jn = sched[ci + 1]
                        if hn == 0 and jn % 8 == 0 and jn > 0:
                            emit_v1T(jn // 8)
                        pa_next = emit_AT(hn, jn)
                    if j % 4 == 0:
                        ptgt_cur[0] = ptgpool.tile(
                            [128, 4, 8, 128], bf16, tag="ptg",
                            name=f"ptg{ci // 4}")
                    emit_mult(h, j, pa, ptgt_cur[0])
                if todo_R and todo_R[0][0] <= ci:
                    _, hh, gg, pt_ = todo_R.pop(0)
                    pb = emit_R(hh, gg, pt_)
                    if EXP_LAG == 0:
                        emit_exp(hh, gg, pb)
                        after_exp(hh, gg)
                    else:
                        todo_E.append((ci + EXP_LAG, hh, gg, pb))
                if todo_E and todo_E[0][0] <= ci:
                    _, hh, gg, pb = todo_E.pop(0)
                    emit_exp(hh, gg, pb)
                    after_exp(hh, gg)
                if ci < len(sched) and sched[ci][1] % 4 == 3:
                    todo_R.append((ci + R_LAG, sched[ci][0],
                                   sched[ci][1] // 4, ptgt_cur[0]))
            emit_zfinal_x2t(1)
            # dummy sqrt: hoists the Sqrt act-table load into the idle ACT
            # window during the tail, off the drain critical path (DMA'd to
            # scratch so dead-code elimination keeps it)
            dum = work.tile([B, 1], f32, tag="dumsq")
            nc.scalar.sqrt(out=dum[:, :], in_=eeT[1][:, 0, 0:1])
            nc.scalar.dma_start(out=dscr_o.ap(), in_=dum[:, :])
            for kt in range(1, 16, 2):
                emit_B(kt, tail=True)

            # ---------- s2 drain + squash per 128-col block ----------
            s2c = work.tile([B, JD], f32, tag="s2c")
            s2 = consts.tile([B, JD], f32, tag="s2")
            v2 = consts.tile([B, JD], f32, tag="v2")
            # ps2 deps are tile-granular, so one full-width copy beats four
            nc.scalar.copy(out=s2c[:, :], in_=ps2[:, :])
            for t in range(4):
                eng = [nc.sync, nc.scalar, nc.sync, nc.scalar][t]
                eng.dma_start(out=cc2i[t].ap(),
                              in_=s2c[:, 128 * t:128 * t + 128])
                allreduce(cc2i[t], cc2o[t], eng=eng)
                eng.dma_start(out=s2[:, 128 * t:128 * t + 128],
                              in_=cc2o[t].ap())
            # single full-width squash once all 4 blocks are in: fewer
            # serialized small DVE ops than a per-block cascade
            _squash_full(nc, work, s2, v2)
            nc.sync.dma_start(out=yout.ap(), in_=v2[:, :])

    nc.compile()
    return nc


_NC_CACHE = None


def _get_nc():
    global _NC_CACHE
    if _NC_CACHE is None:
        _NC_CACHE = build_nc()
    return _NC_CACHE


def _prep_inputs(x, W):
    """Pure layout transform + bf16 cast: returns per-core input dicts."""
    bf = ml_dtypes.bfloat16
    x = np.asarray(x, dtype=np.float32)
    W = np.asarray(W, dtype=np.float32)
    xc = x.reshape(B, NC_, IS, E)             # [b, c, il, e]
    Wc = W[0].reshape(NC_, IS, J, D, E)       # [c, il, j, d, e]
    ident = np.eye(128, dtype=np.float32)
    # r2[(i16*8+e), q, i'] = 1 iff i' == 16q+i16  (e-reduce matrix)
    r2 = np.zeros((128, 8, 128), dtype=np.float32)
    i16 = np.arange(16)
    for q in range(8):
        for e in range(E):
            r2[i16 * 8 + e, q, 16 * q + i16] = 1.0
    outs = []
    for c in range(NC_):
        xv = xc[:, c]                          # [B, 256, 8]
        Wv = Wc[c]                             # [256, J, D, E]
        # xt[lane, kt=(e,half), b] = x[b, half*128+lane, e]
        a = xv.transpose(1, 2, 0).reshape(2, 128, E, B)   # [half, lane, e, b]
        xt = np.ascontiguousarray(a.transpose(1, 2, 0, 3)).reshape(128, 16, B)
        # xt2[(i16*8+e), (h*8+q), b] = x[b, 128h+16q+i16, e]
        a = xv.reshape(B, 2, 8, 16, E)                    # [b, h, q, i16, e]
        xt2 = np.ascontiguousarray(
            a.transpose(3, 4, 1, 2, 0)).reshape(128, 16, B)
        # w1[lane, t, kt, col] = W[il(kt,lane), j, d, e(kt)]/32, (j,d)=128t+col
        a = Wv.reshape(2, 128, J, D, E).transpose(1, 4, 0, 2, 3)
        w1 = np.ascontiguousarray(a).reshape(128, 16, JD) * W1SCALE
        w1 = np.ascontiguousarray(
            w1.reshape(128, 16, 4, 128).transpose(0, 2, 1, 3))
        # wt[32*(j%4)+d, j//4, h, q, (i16*8+e)] = W[128h+16q+i16, j, d, e]
        # (rows 16..31 of each 32-slot are zero: K=32 matmuls, 32-aligned)
        a = Wv.reshape(2, 8, 16, J, D, E)                 # [h, q, i16, j, d, e]
        tmp = a.transpose(4, 3, 0, 1, 2, 5)               # [d, j, h, q, i16, e]
        arr = np.zeros((32, J, 2, 8, 128), dtype=np.float32)
        arr[:16] = tmp.reshape(16, J, 2, 8, 128) * W8SCALE
        wt = np.ascontiguousarray(
            arr.reshape(32, 8, 4, 2, 8, 128).transpose(2, 0, 1, 3, 4, 5)
        ).reshape(128, 8, 2, 8, 128)
        # ws[lane, j, kt, d] = W[il(kt,lane), j, d, e(kt)]
        a = Wv.reshape(2, 128, J, D, E).transpose(1, 2, 4, 0, 3)
        ws = np.ascontiguousarray(a).reshape(128, J, 16, D)
        f8m = ml_dtypes.float8_e4m3fn if USE_F8 else bf
        outs.append({
            "xt": xt.astype(bf), "xt2": xt2.astype(bf),
            "xp8": xt.astype(f8m), "w1": w1.astype(f8m),
            "wt": wt.astype(f8m), "ws": ws.astype(bf), "r2": r2.astype(bf),
            "identf": ident,
        })
    return outs


def run(x, W, trace=False, **kw):
    in_maps = _prep_inputs(x, W)
    nc = _get_nc()
    res = run_bass_kernel_spmd(nc, in_maps, core_ids=list(range(NC_)),
                               trace=trace, **kw)
    out = res.results[0]["yout"].reshape(B, J, D).astype(np.float32)
    return out, res


def kernel(x, W):
    out, _ = run(x, W)
    return out
